# revision 6
# baseline (speedup 1.0000x reference)
"""Trainium2 Bass kernel: linear recurrence cell h_t = Wh h_{t-1} + (x_t W_x^T + b),
outputs (hs * silu(hs), [h0; hs]).

Strategy: data-parallel over batch (B=8 -> 8 cores). Per core, chunked scan:
  - chunks of C=8 steps, M=256 chunks as the matmul free dim (fp32r, full PE rate)
  - phase 1: zero-init local scans L_j = Wh L_{j-1} + W_x x_j + b, batched over chunks
    (input GEMM fused into the same PSUM accumulation); L_j spilled to DRAM
  - carry scan across chunk boundaries: truncated Kogge-Stone doubling with
    host-precomputed Wh^(8s) matrices (bf16; their norms are tiny), rounds chosen
    adaptively from the actual power decay
  - phase 3: Y_j = Wh Y_{j-1} from carries; h_j = L_j + Y_j; out = h * silu(h)
Host does the spectral-norm preprocessing, matrix powers, and layout permutes.
"""

import numpy as np
import ml_dtypes

import concourse.tile as tile
from concourse import bacc, mybir
from concourse.bass_utils import run_bass_kernel_spmd

T, B, D = 2048, 8, 1024
C = 8                  # chunk length (serial steps per phase)
M = T // C             # 256 chunks = matmul free dim
KT = D // 128          # 8 partition tiles over D
PAD = 16               # zero pad columns for shifted KS reads (max shift)
N_CORES = 8
TARGET_RADIUS = np.float32(0.95)
EPS = np.float32(1e-8)

F32R = mybir.dt.float32r
F32 = mybir.dt.float32
BF16 = mybir.dt.bfloat16
ACT = mybir.ActivationFunctionType

_cache = {}
last_results = None


def _spectral_norm_wh(W_h, u):
    """Mirror reference._spectral_norm_wh in float32 numpy."""
    Ws = W_h.astype(np.float32)
    uu = u.astype(np.float32)
    v = None
    for _ in range(3):
        v = Ws.T @ uu
        v = v / (np.linalg.norm(v) + EPS)
        uu = Ws @ v
        uu = uu / (np.linalg.norm(uu) + EPS)
    sigma = np.abs(uu @ W_h @ v)
    return (W_h * (TARGET_RADIUS / (sigma + EPS))).astype(np.float32)


def _build(shifts):
    """Build the SPMD bass program for the given KS shift list."""
    nc = bacc.Bacc("TRN2", target_bir_lowering=False, debug=False,
                   num_devices=N_CORES)

    xp_d = nc.dram_tensor("xp", [C, D, M], F32R, kind="ExternalInput").ap()
    wht_d = nc.dram_tensor("wht", [D, D], F32R, kind="ExternalInput").ap()
    wxt_d = nc.dram_tensor("wxt", [D, D], F32R, kind="ExternalInput").ap()
    ks_d = [nc.dram_tensor(f"ks{s}", [D, D], BF16, kind="ExternalInput").ap()
            for s in shifts]
    h0_d = nc.dram_tensor("h0v", [D], F32R, kind="ExternalInput").ap()
    b_d = nc.dram_tensor("bv", [D], F32, kind="ExternalInput").ap()
    l_d = nc.dram_tensor("lsp", [C, D, M], F32R).ap()  # internal DRAM scratch
    hk_d = nc.dram_tensor("hk", [C, D, M], F32, kind="ExternalOutput").ap()
    ok_d = nc.dram_tensor("ok", [C, D, M], F32, kind="ExternalOutput").ap()

    def wview(ap):  # [D, D] -> [128, KT(k), D(o)]
        return ap.rearrange("(k p) o -> p k o", p=128)

    def cview(ap, j):  # [C, D, M] -> [128, KT, M] for step j
        return ap[j].rearrange("(k p) m -> p k m", p=128)

    with tile.TileContext(nc) as tc:
        with (
            tc.tile_pool(name="wh", bufs=1) as p_wh,
            tc.tile_pool(name="wx", bufs=1) as p_wx,
            tc.tile_pool(name="ks", bufs=2) as p_ks,
            tc.tile_pool(name="lrot", bufs=3) as p_l,
            tc.tile_pool(name="xh", bufs=2) as p_xh,
            tc.tile_pool(name="sil", bufs=4) as p_sil,
            tc.tile_pool(name="pks", bufs=1) as p_pks,
            tc.tile_pool(name="y", bufs=2) as p_y,
            tc.tile_pool(name="small", bufs=1) as p_small,
            tc.tile_pool(name="ps", bufs=8, space="PSUM") as p_ps,
        ):
            wh_sb = p_wh.tile([128, KT, D], F32R)
            nc.sync.dma_start(out=wh_sb, in_=wview(wht_d))
            wx_sb = p_wx.tile([128, KT, D], F32R)
            nc.sync.dma_start(out=wx_sb, in_=wview(wxt_d))
            b_sb = p_small.tile([128, KT], F32)
            nc.sync.dma_start(out=b_sb, in_=b_d.rearrange("(o p) -> p o", p=128))

            # ---- phase 1: local scans, fused input GEMM ----
            L_prev = None
            L_last = None
            for j in range(C):
                X = p_xh.tile([128, KT, M], F32R, tag="x")
                nc.sync.dma_start(out=X, in_=cview(xp_d, j))
                Lj = p_l.tile([128, KT, M], F32R, tag="l")
                for o in range(KT):
                    ps = p_ps.tile([128, M], F32, tag="ps")
                    n_acc = (2 * KT) if j > 0 else KT
                    i = 0
                    if j > 0:
                        for k in range(KT):
                            nc.tensor.matmul(
                                ps, wh_sb[:, k, o * 128:(o + 1) * 128],
                                L_prev[:, k, :],
                                start=(i == 0), stop=(i == n_acc - 1))
                            i += 1
                    for k in range(KT):
                        nc.tensor.matmul(
                            ps, wx_sb[:, k, o * 128:(o + 1) * 128],
                            X[:, k, :],
                            start=(i == 0), stop=(i == n_acc - 1))
                        i += 1
                    nc.scalar.activation(out=Lj[:, o, :], in_=ps,
                                         func=ACT.Identity,
                                         bias=b_sb[:, o:o + 1], scale=1.0)
                nc.sync.dma_start(out=cview(l_d, j), in_=Lj)
                L_prev = Lj
                if j == C - 1:
                    L_last = Lj

            # ---- carry scan: truncated Kogge-Stone ----
            pks = p_pks.tile([128, KT, PAD + M], F32R)
            # f32r pad region is never read (matmuls read the bf16 copy)
            nc.sync.dma_start(out=pks[:, :, PAD],
                              in_=h0_d.rearrange("(k p) -> p k", p=128))
            nc.vector.tensor_copy(out=pks[:, :, PAD + 1:PAD + M],
                                  in_=L_last[:, :, 0:M - 1])
            pks_bf = p_small.tile([128, KT, PAD + M], BF16)
            nc.vector.memset(pks_bf[:, :, 0:PAD], 0.0)
            for r, s in enumerate(shifts):
                mat = p_ks.tile([128, KT, D], BF16, tag="ks")
                nc.sync.dma_start(out=mat, in_=wview(ks_d[r]))
                nc.vector.tensor_copy(out=pks_bf[:, :, PAD:],
                                      in_=pks[:, :, PAD:])
                pss = []
                for o in range(KT):
                    ps = p_ps.tile([128, M], F32, tag="ps")
                    for k in range(KT):
                        nc.tensor.matmul(
                            ps, mat[:, k, o * 128:(o + 1) * 128],
                            pks_bf[:, k, PAD - s:PAD - s + M],
                            start=(k == 0), stop=(k == KT - 1))
                    pss.append(ps)
                for o in range(KT):
                    nc.vector.tensor_add(out=pks[:, o, PAD:],
                                         in0=pks[:, o, PAD:], in1=pss[o])

            # ---- phase 3: propagate carries, finalize outputs ----
            y_prev = pks  # logical columns at [PAD:PAD+M]
            for j in range(C):
                Lj = p_l.tile([128, KT, M], F32R, tag="l")
                nc.sync.dma_start(out=Lj, in_=cview(l_d, j))
                h = p_xh.tile([128, KT, M], F32, tag="x")
                y_new = (p_y.tile([128, KT, M], F32R, tag="y", name=f"ynew{j}")
                         if j < C - 1 else None)
                off = PAD if j == 0 else 0
                for o in range(KT):
                    ps = p_ps.tile([128, M], F32, tag="ps")
                    for k in range(KT):
                        nc.tensor.matmul(
                            ps, wh_sb[:, k, o * 128:(o + 1) * 128],
                            y_prev[:, k, off:off + M],
                            start=(k == 0), stop=(k == KT - 1))
                    if y_new is not None:
                        nc.scalar.activation(out=y_new[:, o, :], in_=ps,
                                             func=ACT.Copy, bias=0.0, scale=1.0)
                    nc.vector.tensor_add(out=h[:, o, :], in0=ps,
                                         in1=Lj[:, o, :])
                    sil = p_sil.tile([128, M], F32, tag="sil")
                    nc.scalar.activation(out=sil, in_=h[:, o, :],
                                         func=ACT.Silu, scale=1.0)
                    nc.vector.tensor_mul(out=sil, in0=h[:, o, :], in1=sil)
                    nc.sync.dma_start(out=cview(ok_d, j)[:, o, :], in_=sil)
                nc.sync.dma_start(out=cview(hk_d, j), in_=h)
                y_prev = y_new

    nc.compile()
    return nc


def kernel(x, h0, W_x, W_h, b, u):
    x = np.ascontiguousarray(x, dtype=np.float32)
    h0 = np.ascontiguousarray(h0, dtype=np.float32)
    W_x = np.ascontiguousarray(W_x, dtype=np.float32)
    W_h = np.ascontiguousarray(W_h, dtype=np.float32)
    b = np.ascontiguousarray(b, dtype=np.float32)
    u = np.ascontiguousarray(u, dtype=np.float32)

    Wh = _spectral_norm_wh(W_h, u)

    # powers for the KS rounds, truncated where decay makes them negligible
    pw = {1: Wh}
    for k in (2, 4, 8, 16, 32, 64, 128):
        pw[k] = (pw[k // 2] @ pw[k // 2]).astype(np.float32)
    shifts = []
    for s in (1, 2, 4, 8, 16):
        shifts.append(s)
        if np.linalg.norm(pw[8 * s]) < 1e-6:
            break

    key = tuple(shifts)
    if key not in _cache:
        _cache[key] = _build(shifts)
    nc = _cache[key]

    wht = np.ascontiguousarray(Wh.T)
    wxt = np.ascontiguousarray(W_x.T)
    ks_mats = {f"ks{s}": np.ascontiguousarray(pw[8 * s].T).astype(ml_dtypes.bfloat16)
               for s in shifts}

    in_maps = []
    for bb in range(B):
        xp = np.ascontiguousarray(
            x[:, bb, :].reshape(M, C, D).transpose(1, 2, 0))
        im = {"xp": xp, "wht": wht, "wxt": wxt,
              "h0v": np.ascontiguousarray(h0[bb]), "bv": b}
        im.update(ks_mats)
        in_maps.append(im)

    import os
    trace = os.environ.get("BASS_KERNEL_TRACE", "0") == "1"
    res = run_bass_kernel_spmd(nc, in_maps, core_ids=list(range(N_CORES)),
                               trace=trace)
    global last_results
    last_results = res

    out = np.empty((T, B, D), np.float32)
    h_all = np.empty((T + 1, B, D), np.float32)
    h_all[0] = h0
    for bb in range(B):
        r = res.results[bb]
        h_all[1:, bb, :] = r["hk"].transpose(2, 0, 1).reshape(T, D)
        out[:, bb, :] = r["ok"].transpose(2, 0, 1).reshape(T, D)
    return out, h_all


if __name__ == "__main__":
    rng = np.random.default_rng(0)
    ins = {
        "x": rng.standard_normal((T, B, D), dtype=np.float32),
        "h0": np.zeros((B, D), np.float32),
        "W_x": (rng.standard_normal((D, D), dtype=np.float32) * 0.02),
        "W_h": (rng.standard_normal((D, D), dtype=np.float32) / np.sqrt(D)),
        "b": np.zeros(D, np.float32),
        "u": rng.standard_normal(D, dtype=np.float32),
    }
    o, ha = kernel(**ins)
    print("ok", o.shape, ha.shape)


# revision 9
# speedup vs baseline: 1.0744x; 1.0744x over previous
"""Trainium2 Bass kernel: linear recurrence cell h_t = Wh h_{t-1} + (x_t W_x^T + b),
outputs (hs * silu(hs), [h0; hs]).

Strategy: data-parallel over batch (B=8 -> 8 cores). Per core, chunked scan:
  - chunks of C=8 steps, M=256 chunks as the matmul free dim (fp32r, full PE rate)
  - phase 1: zero-init local scans L_j = Wh L_{j-1} + W_x x_j + b, batched over chunks
    (input GEMM fused into the same PSUM accumulation); L_j spilled to DRAM
  - carry scan across chunk boundaries: truncated Kogge-Stone doubling with
    host-precomputed Wh^(8s) matrices (bf16; their norms are tiny), rounds chosen
    adaptively from the actual power decay
  - phase 3: Y_j = Wh Y_{j-1} from carries; h_j = L_j + Y_j; out = h * silu(h)
Host does the spectral-norm preprocessing, matrix powers, and layout permutes.
"""

import os

import numpy as np
import ml_dtypes

import concourse.tile as tile
from concourse import bacc, mybir
from concourse.bass_utils import run_bass_kernel_spmd

T, B, D = 2048, 8, 1024
C = 8                  # chunk length (serial steps per phase)
M = T // C             # 256 chunks = matmul free dim
KT = D // 128          # 8 partition tiles over D
PAD = 16               # zero pad columns for shifted KS reads (max shift)
N_CORES = 8
N_WARMUP_MM = 40       # PE warmup matmuls overlapping the initial DMA head
TARGET_RADIUS = np.float32(0.95)
EPS = np.float32(1e-8)

F32R = mybir.dt.float32r
F32 = mybir.dt.float32
BF16 = mybir.dt.bfloat16
ACT = mybir.ActivationFunctionType

_cache = {}
last_results = None


def _spectral_norm_wh(W_h, u):
    """Mirror reference._spectral_norm_wh in float32 numpy."""
    Ws = W_h.astype(np.float32)
    uu = u.astype(np.float32)
    v = None
    for _ in range(3):
        v = Ws.T @ uu
        v = v / (np.linalg.norm(v) + EPS)
        uu = Ws @ v
        uu = uu / (np.linalg.norm(uu) + EPS)
    sigma = np.abs(uu @ W_h @ v)
    return (W_h * (TARGET_RADIUS / (sigma + EPS))).astype(np.float32)


def _build(shifts):
    """Build the SPMD bass program for the given KS shift list."""
    nc = bacc.Bacc("TRN2", target_bir_lowering=False, debug=False,
                   num_devices=N_CORES)

    xp_d = nc.dram_tensor("xp", [C, D, M], F32R, kind="ExternalInput").ap()
    wht_d = nc.dram_tensor("wht", [D, D], F32R, kind="ExternalInput").ap()
    wxt_d = nc.dram_tensor("wxt", [D, D], F32R, kind="ExternalInput").ap()
    ks_d = [nc.dram_tensor(f"ks{s}", [D, D], BF16, kind="ExternalInput").ap()
            for s in shifts]
    h0_d = nc.dram_tensor("h0v", [D], F32R, kind="ExternalInput").ap()
    b_d = nc.dram_tensor("bv", [D], F32, kind="ExternalInput").ap()
    l_d = nc.dram_tensor("lsp", [C, D, M], F32R).ap()  # internal DRAM scratch
    hk_d = nc.dram_tensor("hk", [C, D, M], BF16, kind="ExternalOutput").ap()
    ok_d = nc.dram_tensor("ok", [C, D, M], BF16, kind="ExternalOutput").ap()

    def wview(ap):  # [D, D] -> [128, KT(k), D(o)]
        return ap.rearrange("(k p) o -> p k o", p=128)

    def cview(ap, j):  # [C, D, M] -> [128, KT, M] for step j
        return ap[j].rearrange("(k p) m -> p k m", p=128)

    with tile.TileContext(nc) as tc:
        with (
            tc.tile_pool(name="wh", bufs=1) as p_wh,
            tc.tile_pool(name="wx", bufs=1) as p_wx,
            tc.tile_pool(name="ks", bufs=2) as p_ks,
            tc.tile_pool(name="lrot", bufs=3) as p_l,
            tc.tile_pool(name="xh", bufs=2) as p_xh,
            tc.tile_pool(name="sil", bufs=4) as p_sil,
            tc.tile_pool(name="pks", bufs=1) as p_pks,
            tc.tile_pool(name="y", bufs=2) as p_y,
            tc.tile_pool(name="small", bufs=1) as p_small,
            tc.tile_pool(name="warm", bufs=1) as p_warm,
            tc.tile_pool(name="ps", bufs=8, space="PSUM") as p_ps,
        ):
            # PE warmup on junk data: ramps HAM to full clock during the DMA head
            warm_sb = p_warm.tile([128, 256], BF16)
            nc.vector.memset(warm_sb, 0.0)
            warm_ps = p_ps.tile([128, 256], F32, tag="ps")
            for _ in range(N_WARMUP_MM):
                nc.tensor.matmul(warm_ps, warm_sb[:, 0:128], warm_sb,
                                 start=True, stop=True)

            # DMAs in need order: b, X0, Wx (per o), X1, Wh (per o), KS mats
            b_sb = p_small.tile([128, KT], F32)
            nc.sync.dma_start(out=b_sb, in_=b_d.rearrange("(o p) -> p o", p=128))

            X0 = p_xh.tile([128, KT, M], F32R, tag="x")
            nc.sync.dma_start(out=X0, in_=cview(xp_d, 0))
            wx_sb = p_wx.tile([128, KT, D], F32R)
            for o in range(KT):
                nc.sync.dma_start(out=wx_sb[:, :, o * 128:(o + 1) * 128],
                                  in_=wview(wxt_d)[:, :, o * 128:(o + 1) * 128])
            X1 = p_xh.tile([128, KT, M], F32R, tag="x")
            nc.sync.dma_start(out=X1, in_=cview(xp_d, 1))
            wh_sb = p_wh.tile([128, KT, D], F32R)
            for o in range(KT):
                nc.sync.dma_start(out=wh_sb[:, :, o * 128:(o + 1) * 128],
                                  in_=wview(wht_d)[:, :, o * 128:(o + 1) * 128])
            ks_sb = []
            for r, s in enumerate(shifts):
                mat = p_ks.tile([128, KT, D], BF16, tag="ks", name=f"ksm{s}")
                nc.sync.dma_start(out=mat, in_=wview(ks_d[r]))
                ks_sb.append(mat)

            # ---- phase 1: local scans, fused input GEMM ----
            L_prev = None
            L_last = None
            for j in range(C):
                if j == 0:
                    X = X0
                elif j == 1:
                    X = X1
                else:
                    X = p_xh.tile([128, KT, M], F32R, tag="x", name=f"X{j}")
                    nc.sync.dma_start(out=X, in_=cview(xp_d, j))
                Lj = p_l.tile([128, KT, M], F32R, tag="l", name=f"L{j}")
                for o in range(KT):
                    ps = p_ps.tile([128, M], F32, tag="ps", name=f"ps1_{j}_{o}")
                    n_acc = (2 * KT) if j > 0 else KT
                    i = 0
                    if j > 0:
                        for k in range(KT):
                            nc.tensor.matmul(
                                ps, wh_sb[:, k, o * 128:(o + 1) * 128],
                                L_prev[:, k, :],
                                start=(i == 0), stop=(i == n_acc - 1))
                            i += 1
                    for k in range(KT):
                        nc.tensor.matmul(
                            ps, wx_sb[:, k, o * 128:(o + 1) * 128],
                            X[:, k, :],
                            start=(i == 0), stop=(i == n_acc - 1))
                        i += 1
                    nc.scalar.activation(out=Lj[:, o, :], in_=ps,
                                         func=ACT.Identity,
                                         bias=b_sb[:, o:o + 1], scale=1.0)
                nc.sync.dma_start(out=cview(l_d, j), in_=Lj)
                L_prev = Lj
                if j == C - 1:
                    L_last = Lj

            # ---- carry scan: truncated Kogge-Stone ----
            pks = p_pks.tile([128, KT, PAD + M], F32R)
            # (f32r pad region is never read; matmuls read the bf16 copy)
            nc.sync.dma_start(out=pks[:, :, PAD],
                              in_=h0_d.rearrange("(k p) -> p k", p=128))
            nc.vector.tensor_copy(out=pks[:, :, PAD + 1:PAD + M],
                                  in_=L_last[:, :, 0:M - 1])
            pks_bf = p_small.tile([128, KT, PAD + M], BF16)
            nc.vector.memset(pks_bf[:, :, 0:PAD], 0.0)
            for r, s in enumerate(shifts):
                nc.vector.tensor_copy(out=pks_bf[:, :, PAD:],
                                      in_=pks[:, :, PAD:])
                pss = []
                for o in range(KT):
                    ps = p_ps.tile([128, M], F32, tag="ps", name=f"ps2_{r}_{o}")
                    for k in range(KT):
                        nc.tensor.matmul(
                            ps, ks_sb[r][:, k, o * 128:(o + 1) * 128],
                            pks_bf[:, k, PAD - s:PAD - s + M],
                            start=(k == 0), stop=(k == KT - 1))
                    pss.append(ps)
                for o in range(KT):
                    nc.vector.tensor_add(out=pks[:, o, PAD:],
                                         in0=pks[:, o, PAD:], in1=pss[o])

            # ---- phase 3: propagate carries, finalize outputs ----
            y_prev = pks  # logical columns at [PAD:PAD+M]
            for j in range(C):
                Lj = p_l.tile([128, KT, M], F32R, tag="l", name=f"L3_{j}")
                nc.sync.dma_start(out=Lj, in_=cview(l_d, j))
                h = p_xh.tile([128, KT, M], BF16, tag="x", name=f"h{j}")
                y_new = (p_y.tile([128, KT, M], F32R, tag="y", name=f"ynew{j}")
                         if j < C - 1 else None)
                off = PAD if j == 0 else 0
                for o in range(KT):
                    ps = p_ps.tile([128, M], F32, tag="ps", name=f"ps3_{j}_{o}")
                    for k in range(KT):
                        nc.tensor.matmul(
                            ps, wh_sb[:, k, o * 128:(o + 1) * 128],
                            y_prev[:, k, off:off + M],
                            start=(k == 0), stop=(k == KT - 1))
                    if y_new is not None:
                        nc.scalar.activation(out=y_new[:, o, :], in_=ps,
                                             func=ACT.Copy, bias=0.0, scale=1.0)
                    nc.vector.tensor_add(out=h[:, o, :], in0=ps,
                                         in1=Lj[:, o, :])
                    sil = p_sil.tile([128, M], BF16, tag="sil",
                                     name=f"sil{j}_{o}")
                    nc.scalar.activation(out=sil, in_=h[:, o, :],
                                         func=ACT.Silu, scale=1.0)
                    nc.vector.tensor_mul(out=sil, in0=h[:, o, :], in1=sil)
                    nc.sync.dma_start(out=cview(ok_d, j)[:, o, :], in_=sil)
                nc.sync.dma_start(out=cview(hk_d, j), in_=h)
                y_prev = y_new

    nc.compile()
    return nc


def kernel(x, h0, W_x, W_h, b, u):
    x = np.ascontiguousarray(x, dtype=np.float32)
    h0 = np.ascontiguousarray(h0, dtype=np.float32)
    W_x = np.ascontiguousarray(W_x, dtype=np.float32)
    W_h = np.ascontiguousarray(W_h, dtype=np.float32)
    b = np.ascontiguousarray(b, dtype=np.float32)
    u = np.ascontiguousarray(u, dtype=np.float32)

    Wh = _spectral_norm_wh(W_h, u)

    # powers for the KS rounds, truncated where decay makes them negligible
    pw = {1: Wh}
    for k in (2, 4, 8, 16, 32, 64, 128):
        pw[k] = (pw[k // 2] @ pw[k // 2]).astype(np.float32)
    shifts = []
    for s in (1, 2, 4, 8, 16):
        shifts.append(s)
        if np.linalg.norm(pw[8 * s]) < 1e-6:
            break

    key = tuple(shifts)
    if key not in _cache:
        _cache[key] = _build(shifts)
    nc = _cache[key]

    wht = np.ascontiguousarray(Wh.T)
    wxt = np.ascontiguousarray(W_x.T)
    ks_mats = {f"ks{s}": np.ascontiguousarray(pw[8 * s].T).astype(ml_dtypes.bfloat16)
               for s in shifts}

    in_maps = []
    for bb in range(B):
        xp = np.ascontiguousarray(
            x[:, bb, :].reshape(M, C, D).transpose(1, 2, 0))
        im = {"xp": xp, "wht": wht, "wxt": wxt,
              "h0v": np.ascontiguousarray(h0[bb]), "bv": b}
        im.update(ks_mats)
        in_maps.append(im)

    trace = os.environ.get("BASS_KERNEL_TRACE", "0") == "1"
    res = run_bass_kernel_spmd(nc, in_maps, core_ids=list(range(N_CORES)),
                               trace=trace)
    global last_results
    last_results = res

    out = np.empty((T, B, D), np.float32)
    h_all = np.empty((T + 1, B, D), np.float32)
    h_all[0] = h0
    for bb in range(B):
        r = res.results[bb]
        h_all[1:, bb, :] = (r["hk"].astype(np.float32)
                            .transpose(2, 0, 1).reshape(T, D))
        out[:, bb, :] = (r["ok"].astype(np.float32)
                         .transpose(2, 0, 1).reshape(T, D))
    return out, h_all


if __name__ == "__main__":
    rng = np.random.default_rng(0)
    ins = {
        "x": rng.standard_normal((T, B, D), dtype=np.float32),
        "h0": np.zeros((B, D), np.float32),
        "W_x": (rng.standard_normal((D, D), dtype=np.float32) * 0.02),
        "W_h": (rng.standard_normal((D, D), dtype=np.float32) / np.sqrt(D)),
        "b": np.zeros(D, np.float32),
        "u": rng.standard_normal(D, dtype=np.float32),
    }
    o, ha = kernel(**ins)
    print("ok", o.shape, ha.shape)


# revision 10
# speedup vs baseline: 1.1029x; 1.0265x over previous
"""Trainium2 Bass kernel: linear recurrence cell h_t = Wh h_{t-1} + (x_t W_x^T + b),
outputs (hs * silu(hs), [h0; hs]).

Strategy: data-parallel over batch (B=8 -> 8 cores). Per core, chunked scan:
  - chunks of C=8 steps, M=256 chunks as the matmul free dim (fp32r, full PE rate)
  - phase 1: zero-init local scans L_j = Wh L_{j-1} + W_x x_j + b, batched over chunks
    (input GEMM fused into the same PSUM accumulation); L_j spilled to DRAM
  - carry scan across chunk boundaries: truncated Kogge-Stone doubling with
    host-precomputed Wh^(8s) matrices (bf16); rounds chosen adaptively from the
    actual power decay (the spectral radius of Wh is ~0.5, so memory is ~32 steps)
  - phase 3: Y_j = Wh Y_{j-1} from carries; h_j = L_j + Y_j (fp32), then
    out = h * silu(h); h and out are written as bf16, two steps per DMA so DRAM
    lines stay >= 1KB
Host does the spectral-norm preprocessing, matrix powers, and layout permutes.
"""

import os

import numpy as np
import ml_dtypes

import concourse.tile as tile
from concourse import bacc, mybir
from concourse.bass_utils import run_bass_kernel_spmd

T, B, D = 2048, 8, 1024
C = 8                  # chunk length (serial steps per phase)
M = T // C             # 256 chunks = matmul free dim
KT = D // 128          # 8 partition tiles over D
PAD = 16               # zero pad columns for shifted KS reads (max shift)
N_CORES = 8
N_WARMUP_MM = 40       # PE warmup matmuls overlapping the initial DMA head
KS_TOL = 1e-3          # drop KS rounds whose matrix norm is below this
TARGET_RADIUS = np.float32(0.95)
EPS = np.float32(1e-8)

F32R = mybir.dt.float32r
F32 = mybir.dt.float32
BF16 = mybir.dt.bfloat16
ACT = mybir.ActivationFunctionType

_cache = {}
last_results = None


def _spectral_norm_wh(W_h, u):
    """Mirror reference._spectral_norm_wh in float32 numpy."""
    Ws = W_h.astype(np.float32)
    uu = u.astype(np.float32)
    v = None
    for _ in range(3):
        v = Ws.T @ uu
        v = v / (np.linalg.norm(v) + EPS)
        uu = Ws @ v
        uu = uu / (np.linalg.norm(uu) + EPS)
    sigma = np.abs(uu @ W_h @ v)
    return (W_h * (TARGET_RADIUS / (sigma + EPS))).astype(np.float32)


def _build(shifts):
    """Build the SPMD bass program for the given KS shift list."""
    nc = bacc.Bacc("TRN2", target_bir_lowering=False, debug=False,
                   num_devices=N_CORES)

    xp_d = nc.dram_tensor("xp", [C, D, M], F32R, kind="ExternalInput").ap()
    wht_d = nc.dram_tensor("wht", [D, D], F32R, kind="ExternalInput").ap()
    wxt_d = nc.dram_tensor("wxt", [D, D], F32R, kind="ExternalInput").ap()
    ks_d = [nc.dram_tensor(f"ks{s}", [D, D], BF16, kind="ExternalInput").ap()
            for s in shifts]
    h0_d = nc.dram_tensor("h0v", [D], F32R, kind="ExternalInput").ap()
    b_d = nc.dram_tensor("bv", [D], F32, kind="ExternalInput").ap()
    l_d = nc.dram_tensor("lsp", [C, D, M], F32R).ap()  # internal DRAM scratch
    hk_d = nc.dram_tensor("hk", [C // 2, D, 2 * M], BF16,
                          kind="ExternalOutput").ap()
    ok_d = nc.dram_tensor("ok", [C // 2, D, 2 * M], BF16,
                          kind="ExternalOutput").ap()

    def wview(ap):  # [D, D] -> [128, KT(k), D(o)]
        return ap.rearrange("(k p) o -> p k o", p=128)

    def cview(ap, j):  # [C, D, M] -> [128, KT, M] for step j
        return ap[j].rearrange("(k p) m -> p k m", p=128)

    def pview(ap, jp):  # [C/2, D, 2M] -> [128, KT, 2M] for step pair jp
        return ap[jp].rearrange("(k p) m -> p k m", p=128)

    with tile.TileContext(nc) as tc:
        with (
            tc.tile_pool(name="wh", bufs=1) as p_wh,
            tc.tile_pool(name="wx", bufs=1) as p_wx,
            tc.tile_pool(name="ks", bufs=1) as p_ks,
            tc.tile_pool(name="lrot", bufs=3) as p_l,
            tc.tile_pool(name="xh", bufs=4) as p_xh,
            tc.tile_pool(name="sil", bufs=4) as p_sil,
            tc.tile_pool(name="hf", bufs=4) as p_hf,
            tc.tile_pool(name="pks", bufs=1) as p_pks,
            tc.tile_pool(name="y", bufs=2) as p_y,
            tc.tile_pool(name="small", bufs=1) as p_small,
            tc.tile_pool(name="warm", bufs=1) as p_warm,
            tc.tile_pool(name="ps", bufs=8, space="PSUM") as p_ps,
        ):
            # PE warmup on junk data: ramps HAM to full clock during the DMA head
            warm_sb = p_warm.tile([128, 256], BF16)
            nc.vector.memset(warm_sb, 0.0)
            warm_ps = p_ps.tile([128, 256], F32, tag="ps")
            for _ in range(N_WARMUP_MM):
                nc.tensor.matmul(warm_ps, warm_sb[:, 0:128], warm_sb,
                                 start=True, stop=True)

            # DMAs in need order: b, X0, Wx (per o), X1, Wh (per o)
            b_sb = p_small.tile([128, KT], F32)
            nc.sync.dma_start(out=b_sb, in_=b_d.rearrange("(o p) -> p o", p=128))

            X0 = p_xh.tile([128, KT, M], F32R, tag="x")
            nc.sync.dma_start(out=X0, in_=cview(xp_d, 0))
            wx_sb = p_wx.tile([128, KT, D], F32R)
            for o in range(KT):
                nc.sync.dma_start(out=wx_sb[:, :, o * 128:(o + 1) * 128],
                                  in_=wview(wxt_d)[:, :, o * 128:(o + 1) * 128])
            X1 = p_xh.tile([128, KT, M], F32R, tag="x")
            nc.sync.dma_start(out=X1, in_=cview(xp_d, 1))
            wh_sb = p_wh.tile([128, KT, D], F32R)
            for o in range(KT):
                nc.sync.dma_start(out=wh_sb[:, :, o * 128:(o + 1) * 128],
                                  in_=wview(wht_d)[:, :, o * 128:(o + 1) * 128])

            # ---- phase 1: local scans, fused input GEMM ----
            ks_sb = []
            L_prev = None
            L_last = None
            for j in range(C):
                if j == 0:
                    X = X0
                elif j == 1:
                    X = X1
                else:
                    X = p_xh.tile([128, KT, M], F32R, tag="x", name=f"X{j}")
                    nc.sync.dma_start(out=X, in_=cview(xp_d, j))
                Lj = p_l.tile([128, KT, M], F32R, tag="l", name=f"L{j}")
                for o in range(KT):
                    ps = p_ps.tile([128, M], F32, tag="ps", name=f"ps1_{j}_{o}")
                    n_acc = (2 * KT) if j > 0 else KT
                    i = 0
                    if j > 0:
                        for k in range(KT):
                            nc.tensor.matmul(
                                ps, wh_sb[:, k, o * 128:(o + 1) * 128],
                                L_prev[:, k, :],
                                start=(i == 0), stop=(i == n_acc - 1))
                            i += 1
                    for k in range(KT):
                        nc.tensor.matmul(
                            ps, wx_sb[:, k, o * 128:(o + 1) * 128],
                            X[:, k, :],
                            start=(i == 0), stop=(i == n_acc - 1))
                        i += 1
                    nc.scalar.activation(out=Lj[:, o, :], in_=ps,
                                         func=ACT.Identity,
                                         bias=b_sb[:, o:o + 1], scale=1.0)
                nc.sync.dma_start(out=cview(l_d, j), in_=Lj)
                if j == 3:  # late enough not to starve X prefetches
                    for r, s in enumerate(shifts):
                        mat = p_ks.tile([128, KT, D], BF16, tag="ks",
                                        name=f"ksm{s}")
                        nc.sync.dma_start(out=mat, in_=wview(ks_d[r]))
                        ks_sb.append(mat)
                L_prev = Lj
                if j == C - 1:
                    L_last = Lj

            # ---- carry scan: truncated Kogge-Stone ----
            pks = p_pks.tile([128, KT, PAD + M], F32R)
            # (f32r pad region is never read; matmuls read the bf16 copy)
            nc.sync.dma_start(out=pks[:, :, PAD],
                              in_=h0_d.rearrange("(k p) -> p k", p=128))
            nc.vector.tensor_copy(out=pks[:, :, PAD + 1:PAD + M],
                                  in_=L_last[:, :, 0:M - 1])
            pks_bf = p_small.tile([128, KT, PAD + M], BF16)
            nc.vector.memset(pks_bf[:, :, 0:PAD], 0.0)
            for r, s in enumerate(shifts):
                nc.vector.tensor_copy(out=pks_bf[:, :, PAD:],
                                      in_=pks[:, :, PAD:])
                pss = []
                for o in range(KT):
                    ps = p_ps.tile([128, M], F32, tag="ps", name=f"ps2_{r}_{o}")
                    for k in range(KT):
                        nc.tensor.matmul(
                            ps, ks_sb[r][:, k, o * 128:(o + 1) * 128],
                            pks_bf[:, k, PAD - s:PAD - s + M],
                            start=(k == 0), stop=(k == KT - 1))
                    pss.append(ps)
                for o in range(KT):
                    nc.vector.tensor_add(out=pks[:, o, PAD:],
                                         in0=pks[:, o, PAD:], in1=pss[o])

            # ---- phase 3: propagate carries, finalize outputs ----
            y_prev = pks  # logical columns at [PAD:PAD+M]
            h_pair = None
            o_pair = None
            for j in range(C):
                jj = j % 2
                Lj = p_l.tile([128, KT, M], F32R, tag="l", name=f"L3_{j}")
                nc.sync.dma_start(out=Lj, in_=cview(l_d, j))
                if jj == 0:
                    h_pair = p_xh.tile([128, KT, 2, M], BF16, tag="x",
                                       name=f"hp{j}")
                    o_pair = p_xh.tile([128, KT, 2, M], BF16, tag="x",
                                       name=f"op{j}")
                y_new = (p_y.tile([128, KT, M], F32R, tag="y", name=f"ynew{j}")
                         if j < C - 1 else None)
                off = PAD if j == 0 else 0
                for o in range(KT):
                    ps = p_ps.tile([128, M], F32, tag="ps", name=f"ps3_{j}_{o}")
                    for k in range(KT):
                        nc.tensor.matmul(
                            ps, wh_sb[:, k, o * 128:(o + 1) * 128],
                            y_prev[:, k, off:off + M],
                            start=(k == 0), stop=(k == KT - 1))
                    if y_new is not None:
                        nc.vector.tensor_copy(out=y_new[:, o, :], in_=ps)
                    hf = p_hf.tile([128, M], F32, tag="hf", name=f"hf{j}_{o}")
                    nc.vector.tensor_add(out=hf, in0=ps, in1=Lj[:, o, :])
                    sil = p_sil.tile([128, M], BF16, tag="sil",
                                     name=f"sil{j}_{o}")
                    nc.scalar.activation(out=sil, in_=hf, func=ACT.Silu,
                                         scale=1.0)
                    nc.vector.tensor_mul(out=o_pair[:, o, jj, :], in0=hf,
                                         in1=sil)
                    nc.scalar.activation(out=h_pair[:, o, jj, :], in_=hf,
                                         func=ACT.Copy, bias=0.0, scale=1.0)
                if jj == 1:
                    nc.sync.dma_start(out=pview(hk_d, j // 2), in_=h_pair)
                    nc.sync.dma_start(out=pview(ok_d, j // 2), in_=o_pair)
                y_prev = y_new

    nc.compile()
    return nc


def kernel(x, h0, W_x, W_h, b, u):
    x = np.ascontiguousarray(x, dtype=np.float32)
    h0 = np.ascontiguousarray(h0, dtype=np.float32)
    W_x = np.ascontiguousarray(W_x, dtype=np.float32)
    W_h = np.ascontiguousarray(W_h, dtype=np.float32)
    b = np.ascontiguousarray(b, dtype=np.float32)
    u = np.ascontiguousarray(u, dtype=np.float32)

    Wh = _spectral_norm_wh(W_h, u)

    # powers for the KS rounds, truncated where decay makes them negligible
    pw = {1: Wh}
    for k in (2, 4, 8, 16, 32, 64, 128):
        pw[k] = (pw[k // 2] @ pw[k // 2]).astype(np.float32)
    shifts = []
    for s in (1, 2, 4, 8, 16):
        if np.linalg.norm(pw[8 * s]) < KS_TOL:
            break
        shifts.append(s)
    if not shifts:
        shifts = [1]

    key = tuple(shifts)
    if key not in _cache:
        _cache[key] = _build(shifts)
    nc = _cache[key]

    wht = np.ascontiguousarray(Wh.T)
    wxt = np.ascontiguousarray(W_x.T)
    ks_mats = {f"ks{s}": np.ascontiguousarray(pw[8 * s].T).astype(ml_dtypes.bfloat16)
               for s in shifts}

    in_maps = []
    for bb in range(B):
        xp = np.ascontiguousarray(
            x[:, bb, :].reshape(M, C, D).transpose(1, 2, 0))
        im = {"xp": xp, "wht": wht, "wxt": wxt,
              "h0v": np.ascontiguousarray(h0[bb]), "bv": b}
        im.update(ks_mats)
        in_maps.append(im)

    trace = os.environ.get("BASS_KERNEL_TRACE", "0") == "1"
    res = run_bass_kernel_spmd(nc, in_maps, core_ids=list(range(N_CORES)),
                               trace=trace)
    global last_results
    last_results = res

    out = np.empty((T, B, D), np.float32)
    h_all = np.empty((T + 1, B, D), np.float32)
    h_all[0] = h0
    for bb in range(B):
        r = res.results[bb]
        # [C/2, D, 2, M] -> [M, C/2, 2, D] -> [T, D]  (t = m*C + jp*2 + jj)
        hk = r["hk"].astype(np.float32).reshape(C // 2, D, 2, M)
        ok = r["ok"].astype(np.float32).reshape(C // 2, D, 2, M)
        h_all[1:, bb, :] = hk.transpose(3, 0, 2, 1).reshape(T, D)
        out[:, bb, :] = ok.transpose(3, 0, 2, 1).reshape(T, D)
    return out, h_all


if __name__ == "__main__":
    rng = np.random.default_rng(0)
    ins = {
        "x": rng.standard_normal((T, B, D), dtype=np.float32),
        "h0": np.zeros((B, D), np.float32),
        "W_x": (rng.standard_normal((D, D), dtype=np.float32) * 0.02),
        "W_h": (rng.standard_normal((D, D), dtype=np.float32) / np.sqrt(D)),
        "b": np.zeros(D, np.float32),
        "u": rng.standard_normal(D, dtype=np.float32),
    }
    o, ha = kernel(**ins)
    print("ok", o.shape, ha.shape)


# revision 12
# speedup vs baseline: 1.1234x; 1.0186x over previous
"""Trainium2 Bass kernel: linear recurrence cell h_t = Wh h_{t-1} + (x_t W_x^T + b),
outputs (hs * silu(hs), [h0; hs]).

Strategy: data-parallel over batch (B=8 -> 8 cores). Per core, chunked scan:
  - chunks of C=8 steps, M=256 chunks as the matmul free dim (fp32r, full PE rate)
  - phase 1: zero-init local scans L_j = Wh L_{j-1} + W_x x_j + b, batched over chunks
    (input GEMM fused into the same PSUM accumulation); L_j spilled to DRAM
  - carry scan across chunk boundaries: truncated Kogge-Stone doubling with
    host-precomputed Wh^(8s) matrices (bf16); rounds chosen adaptively from the
    actual power decay (the spectral radius of Wh is ~0.5, so memory is ~32 steps)
  - phase 3: Y_j = Wh Y_{j-1} from carries; h_j = L_j + Y_j (fp32), then
    out = h * silu(h); h and out are written as bf16, two steps per DMA so DRAM
    lines stay >= 1KB
Host does the spectral-norm preprocessing, matrix powers, and layout permutes.
"""

import os

import numpy as np
import ml_dtypes

import concourse.tile as tile
from concourse import bacc, mybir
from concourse.bass_utils import run_bass_kernel_spmd

T, B, D = 2048, 8, 1024
C = 8                  # chunk length (serial steps per phase)
M = T // C             # 256 chunks = matmul free dim
KT = D // 128          # 8 partition tiles over D
PAD = 16               # zero pad columns for shifted KS reads (max shift)
N_CORES = 8
N_WARMUP_MM = 56       # PE warmup matmuls overlapping the initial DMA head
KS_TOL = 1e-3          # drop KS rounds whose matrix norm is below this
TARGET_RADIUS = np.float32(0.95)
EPS = np.float32(1e-8)

F32R = mybir.dt.float32r
F32 = mybir.dt.float32
BF16 = mybir.dt.bfloat16
ACT = mybir.ActivationFunctionType

_cache = {}
last_results = None


def _spectral_norm_wh(W_h, u):
    """Mirror reference._spectral_norm_wh in float32 numpy."""
    Ws = W_h.astype(np.float32)
    uu = u.astype(np.float32)
    v = None
    for _ in range(3):
        v = Ws.T @ uu
        v = v / (np.linalg.norm(v) + EPS)
        uu = Ws @ v
        uu = uu / (np.linalg.norm(uu) + EPS)
    sigma = np.abs(uu @ W_h @ v)
    return (W_h * (TARGET_RADIUS / (sigma + EPS))).astype(np.float32)


def _build(shifts):
    """Build the SPMD bass program for the given KS shift list."""
    nc = bacc.Bacc("TRN2", target_bir_lowering=False, debug=False,
                   num_devices=N_CORES)

    xp_d = nc.dram_tensor("xp", [C, D, M], F32R, kind="ExternalInput").ap()
    wht_d = nc.dram_tensor("wht", [D, D], F32R, kind="ExternalInput").ap()
    wxt_d = nc.dram_tensor("wxt", [D, D], F32R, kind="ExternalInput").ap()
    ks_d = [nc.dram_tensor(f"ks{s}", [D, D], BF16, kind="ExternalInput").ap()
            for s in shifts]
    h0_d = nc.dram_tensor("h0v", [D], F32R, kind="ExternalInput").ap()
    b_d = nc.dram_tensor("bv", [D], F32, kind="ExternalInput").ap()
    l_d = nc.dram_tensor("lsp", [C, D, M], F32R).ap()  # internal DRAM scratch
    hk_d = nc.dram_tensor("hk", [C // 2, D, 2 * M], BF16,
                          kind="ExternalOutput").ap()
    ok_d = nc.dram_tensor("ok", [C // 2, D, 2 * M], BF16,
                          kind="ExternalOutput").ap()

    def wview(ap):  # [D, D] -> [128, KT(k), D(o)]
        return ap.rearrange("(k p) o -> p k o", p=128)

    def cview(ap, j):  # [C, D, M] -> [128, KT, M] for step j
        return ap[j].rearrange("(k p) m -> p k m", p=128)

    def pview(ap, jp):  # [C/2, D, 2M] -> [128, KT, 2M] for step pair jp
        return ap[jp].rearrange("(k p) m -> p k m", p=128)

    with tile.TileContext(nc) as tc:
        with (
            tc.tile_pool(name="wh", bufs=1) as p_wh,
            tc.tile_pool(name="wx", bufs=1) as p_wx,
            tc.tile_pool(name="ks", bufs=1) as p_ks,
            tc.tile_pool(name="lrot", bufs=3) as p_l,
            tc.tile_pool(name="xh", bufs=4) as p_xh,
            tc.tile_pool(name="sil", bufs=4) as p_sil,
            tc.tile_pool(name="hf", bufs=4) as p_hf,
            tc.tile_pool(name="pks", bufs=1) as p_pks,
            tc.tile_pool(name="y", bufs=2) as p_y,
            tc.tile_pool(name="small", bufs=1) as p_small,
            tc.tile_pool(name="warm", bufs=1) as p_warm,
            tc.tile_pool(name="ps", bufs=8, space="PSUM") as p_ps,
        ):
            # PE warmup on junk data: ramps HAM to full clock during the DMA head
            warm_sb = p_warm.tile([128, 256], BF16)
            nc.vector.memset(warm_sb, 0.0)
            warm_ps = p_ps.tile([128, 256], F32, tag="ps")
            for _ in range(N_WARMUP_MM):
                nc.tensor.matmul(warm_ps, warm_sb[:, 0:128], warm_sb,
                                 start=True, stop=True)

            # DMAs in need order: b, X0, Wx (per o), X1, Wh (per o)
            b_sb = p_small.tile([128, KT], F32)
            nc.sync.dma_start(out=b_sb, in_=b_d.rearrange("(o p) -> p o", p=128))
            h0_sb = p_small.tile([128, KT], F32R)
            nc.sync.dma_start(out=h0_sb, in_=h0_d.rearrange("(k p) -> p k", p=128))

            X0 = p_xh.tile([128, KT, M], F32R, tag="x")
            nc.sync.dma_start(out=X0, in_=cview(xp_d, 0))
            wx_sb = p_wx.tile([128, KT, D], F32R)
            for o in range(KT):
                nc.sync.dma_start(out=wx_sb[:, :, o * 128:(o + 1) * 128],
                                  in_=wview(wxt_d)[:, :, o * 128:(o + 1) * 128])
            X1 = p_xh.tile([128, KT, M], F32R, tag="x")
            nc.sync.dma_start(out=X1, in_=cview(xp_d, 1))
            wh_sb = p_wh.tile([128, KT, D], F32R)
            for o in range(KT):
                nc.sync.dma_start(out=wh_sb[:, :, o * 128:(o + 1) * 128],
                                  in_=wview(wht_d)[:, :, o * 128:(o + 1) * 128])

            # ---- phase 1: local scans, fused input GEMM ----
            ks_sb = []
            L_prev = None
            L_last = None
            for j in range(C):
                if j == 0:
                    X = X0
                elif j == 1:
                    X = X1
                else:
                    X = p_xh.tile([128, KT, M], F32R, tag="x", name=f"X{j}")
                    nc.sync.dma_start(out=X, in_=cview(xp_d, j))
                Lj = p_l.tile([128, KT, M], F32R, tag="l", name=f"L{j}")
                for o in range(KT):
                    ps = p_ps.tile([128, M], F32, tag="ps", name=f"ps1_{j}_{o}")
                    n_acc = (2 * KT) if j > 0 else KT
                    i = 0
                    if j > 0:
                        for k in range(KT):
                            nc.tensor.matmul(
                                ps, wh_sb[:, k, o * 128:(o + 1) * 128],
                                L_prev[:, k, :],
                                start=(i == 0), stop=(i == n_acc - 1))
                            i += 1
                    for k in range(KT):
                        nc.tensor.matmul(
                            ps, wx_sb[:, k, o * 128:(o + 1) * 128],
                            X[:, k, :],
                            start=(i == 0), stop=(i == n_acc - 1))
                        i += 1
                    nc.scalar.activation(out=Lj[:, o, :], in_=ps,
                                         func=ACT.Identity,
                                         bias=b_sb[:, o:o + 1], scale=1.0)
                nc.gpsimd.dma_start(out=cview(l_d, j), in_=Lj)
                if j == 5:  # late enough not to starve X prefetches
                    for r, s in enumerate(shifts):
                        mat = p_ks.tile([128, KT, D], BF16, tag="ks",
                                        name=f"ksm{s}")
                        nc.sync.dma_start(out=mat, in_=wview(ks_d[r]))
                        ks_sb.append(mat)
                L_prev = Lj
                if j == C - 1:
                    L_last = Lj

            # ---- carry scan: truncated Kogge-Stone ----
            pks = p_pks.tile([128, KT, PAD + M], F32R)
            # (f32r pad region is never read; matmuls read the bf16 copy)
            pks_bf = p_small.tile([128, KT, PAD + M], BF16)
            nc.vector.memset(pks_bf[:, :, 0:PAD], 0.0)
            nc.vector.tensor_copy(out=pks_bf[:, :, PAD + 1:PAD + M],
                                  in_=L_last[:, :, 0:M - 1])
            nc.vector.tensor_copy(out=pks_bf[:, :, PAD], in_=h0_sb)
            nc.vector.tensor_copy(out=pks[:, :, PAD], in_=h0_sb)
            nc.vector.tensor_copy(out=pks[:, :, PAD + 1:PAD + M],
                                  in_=L_last[:, :, 0:M - 1])
            for r, s in enumerate(shifts):
                if r > 0:
                    nc.vector.tensor_copy(out=pks_bf[:, :, PAD:],
                                          in_=pks[:, :, PAD:])
                pss = []
                for o in range(KT):
                    ps = p_ps.tile([128, M], F32, tag="ps", name=f"ps2_{r}_{o}")
                    for k in range(KT):
                        nc.tensor.matmul(
                            ps, ks_sb[r][:, k, o * 128:(o + 1) * 128],
                            pks_bf[:, k, PAD - s:PAD - s + M],
                            start=(k == 0), stop=(k == KT - 1))
                    pss.append(ps)
                for o in range(KT):
                    nc.vector.tensor_add(out=pks[:, o, PAD:],
                                         in0=pks[:, o, PAD:], in1=pss[o])

            # ---- phase 3: propagate carries, finalize outputs ----
            y_prev = pks  # logical columns at [PAD:PAD+M]
            h_pair = None
            o_pair = None
            for j in range(C):
                jj = j % 2
                Lj = p_l.tile([128, KT, M], F32R, tag="l", name=f"L3_{j}")
                nc.sync.dma_start(out=Lj, in_=cview(l_d, j))
                if jj == 0:
                    h_pair = p_xh.tile([128, KT, 2, M], BF16, tag="x",
                                       name=f"hp{j}")
                    o_pair = p_xh.tile([128, KT, 2, M], BF16, tag="x",
                                       name=f"op{j}")
                y_new = (p_y.tile([128, KT, M], F32R, tag="y", name=f"ynew{j}")
                         if j < C - 1 else None)
                off = PAD if j == 0 else 0
                for o in range(KT):
                    ps = p_ps.tile([128, M], F32, tag="ps", name=f"ps3_{j}_{o}")
                    for k in range(KT):
                        nc.tensor.matmul(
                            ps, wh_sb[:, k, o * 128:(o + 1) * 128],
                            y_prev[:, k, off:off + M],
                            start=(k == 0), stop=(k == KT - 1))
                    if y_new is not None:
                        nc.vector.tensor_copy(out=y_new[:, o, :], in_=ps)
                    hf = p_hf.tile([128, M], F32, tag="hf", name=f"hf{j}_{o}")
                    nc.vector.tensor_add(out=hf, in0=ps, in1=Lj[:, o, :])
                    sil = p_sil.tile([128, M], BF16, tag="sil",
                                     name=f"sil{j}_{o}")
                    nc.scalar.activation(out=sil, in_=hf, func=ACT.Silu,
                                         scale=1.0)
                    nc.vector.tensor_mul(out=o_pair[:, o, jj, :], in0=hf,
                                         in1=sil)
                    nc.scalar.activation(out=h_pair[:, o, jj, :], in_=hf,
                                         func=ACT.Copy, bias=0.0, scale=1.0)
                if jj == 1:
                    if j == C - 1:
                        for o in range(KT):
                            nc.gpsimd.dma_start(
                                out=pview(hk_d, j // 2)[:, o, :],
                                in_=h_pair[:, o, :, :].rearrange(
                                    "p two m -> p (two m)"))
                            nc.gpsimd.dma_start(
                                out=pview(ok_d, j // 2)[:, o, :],
                                in_=o_pair[:, o, :, :].rearrange(
                                    "p two m -> p (two m)"))
                    else:
                        nc.gpsimd.dma_start(out=pview(hk_d, j // 2), in_=h_pair)
                        nc.gpsimd.dma_start(out=pview(ok_d, j // 2), in_=o_pair)
                y_prev = y_new

    nc.compile()
    return nc


def kernel(x, h0, W_x, W_h, b, u):
    x = np.ascontiguousarray(x, dtype=np.float32)
    h0 = np.ascontiguousarray(h0, dtype=np.float32)
    W_x = np.ascontiguousarray(W_x, dtype=np.float32)
    W_h = np.ascontiguousarray(W_h, dtype=np.float32)
    b = np.ascontiguousarray(b, dtype=np.float32)
    u = np.ascontiguousarray(u, dtype=np.float32)

    Wh = _spectral_norm_wh(W_h, u)

    # powers for the KS rounds, truncated where decay makes them negligible
    pw = {1: Wh}
    for k in (2, 4, 8, 16, 32, 64, 128):
        pw[k] = (pw[k // 2] @ pw[k // 2]).astype(np.float32)
    shifts = []
    for s in (1, 2, 4, 8, 16):
        if np.linalg.norm(pw[8 * s]) < KS_TOL:
            break
        shifts.append(s)
    if not shifts:
        shifts = [1]

    key = tuple(shifts)
    if key not in _cache:
        _cache[key] = _build(shifts)
    nc = _cache[key]

    wht = np.ascontiguousarray(Wh.T)
    wxt = np.ascontiguousarray(W_x.T)
    ks_mats = {f"ks{s}": np.ascontiguousarray(pw[8 * s].T).astype(ml_dtypes.bfloat16)
               for s in shifts}

    in_maps = []
    for bb in range(B):
        xp = np.ascontiguousarray(
            x[:, bb, :].reshape(M, C, D).transpose(1, 2, 0))
        im = {"xp": xp, "wht": wht, "wxt": wxt,
              "h0v": np.ascontiguousarray(h0[bb]), "bv": b}
        im.update(ks_mats)
        in_maps.append(im)

    trace = os.environ.get("BASS_KERNEL_TRACE", "0") == "1"
    res = run_bass_kernel_spmd(nc, in_maps, core_ids=list(range(N_CORES)),
                               trace=trace)
    global last_results
    last_results = res

    out = np.empty((T, B, D), np.float32)
    h_all = np.empty((T + 1, B, D), np.float32)
    h_all[0] = h0
    for bb in range(B):
        r = res.results[bb]
        # [C/2, D, 2, M] -> [M, C/2, 2, D] -> [T, D]  (t = m*C + jp*2 + jj)
        hk = r["hk"].astype(np.float32).reshape(C // 2, D, 2, M)
        ok = r["ok"].astype(np.float32).reshape(C // 2, D, 2, M)
        h_all[1:, bb, :] = hk.transpose(3, 0, 2, 1).reshape(T, D)
        out[:, bb, :] = ok.transpose(3, 0, 2, 1).reshape(T, D)
    return out, h_all


if __name__ == "__main__":
    rng = np.random.default_rng(0)
    ins = {
        "x": rng.standard_normal((T, B, D), dtype=np.float32),
        "h0": np.zeros((B, D), np.float32),
        "W_x": (rng.standard_normal((D, D), dtype=np.float32) * 0.02),
        "W_h": (rng.standard_normal((D, D), dtype=np.float32) / np.sqrt(D)),
        "b": np.zeros(D, np.float32),
        "u": rng.standard_normal(D, dtype=np.float32),
    }
    o, ha = kernel(**ins)
    print("ok", o.shape, ha.shape)
